# revision 1
# baseline (speedup 1.0000x reference)
"""CapsuleLayer (dynamic routing) Trainium2 kernel.

Math (per example a):
  H[a,b,c,j] = sum_i x[a,c,i] * W[b,c,j,i]          (inputs_hat)
  3 routing iterations of:
    coef = softmax_b(L); s = sum_c coef*H; out = squash(s); L += sum_d out*H

Distribution: data-parallel over batch, 512 = 8 cores x 64 examples.

Per-core layout: SBUF partition p = b0*64 + a  (b0 = capsule_half, a = local
example), so the routing loop is fully partition-parallel; only the softmax
denominator needs a tiny cross-partition (p <-> p+64) fixup via SBUF DMA.

H is generated on the tensor engine with the *inputs* as the stationary
operand (M = examples -> PSUM partitions already match the routing layout).
K=8 contraction is packed 4 row-tiles (4 c values) x 2 col-tiles (b halves)
via tile_position so 8 small matmuls run concurrently in the PE array.

H is stored bf16 [128, (b16, d16, c144)]: every big routing op then runs in
DVE 2x mode except the b-update multiply (broadcast on innermost axis).
"""

import os
import sys

for _p in ("/opt/trn_rl_repo",):
    if _p not in sys.path:
        sys.path.insert(0, _p)

from contextlib import ExitStack

import numpy as np

import concourse.bass as bass
import concourse.mybir as mybir
from concourse import tile
from concourse.bass_utils import run_bass_kernel_spmd

F32 = mybir.dt.float32
BF16 = mybir.dt.bfloat16
AF = mybir.ActivationFunctionType
ALU = mybir.AluOpType
AX = mybir.AxisListType

B = 512
NCORES = 8
BS = B // NCORES  # 64 examples per core
NCAP = 32
B16 = 16  # capsules per half
CIN = 144
CQ = 36  # c // 4
D = 16
I8 = 8
EPS = 1e-7
ROUTINGS = 3

HFREE = B16 * D * CIN  # 36864 elements per partition


def _build_program() -> bass.Bass:
    nc = bass.Bass()
    ilhs_d = nc.declare_dram_parameter("ilhs", [32, CQ * BS], BF16, isOutput=False)
    wrhs_d = nc.declare_dram_parameter("wrhs", [32, CQ * 512], BF16, isOutput=False)
    out_d = nc.declare_dram_parameter("out", [BS, NCAP, D], F32, isOutput=True)

    with ExitStack() as ctx:
        tc = ctx.enter_context(tile.TileContext(nc))
        cpool = ctx.enter_context(tc.tile_pool(name="const", bufs=1))

        H1a = cpool.tile([128, B16 * D * 72], BF16)  # c in [0, 72)
        H1b = cpool.tile([128, B16 * D * 72], BF16)  # c in [72, 144)
        # scratch for the big elementwise passes; allocated up front so
        # its space never aliases the W-streaming pool (an alias adds a
        # WAR dependency that delays the first routing ops until every
        # matmul has consumed its W chunk)
        prod = cpool.tile([128, HFREE], BF16)
        ilhs_t = cpool.tile([128, CQ * BS], BF16)

        # persistent small tensors
        s_t = cpool.tile([128, 256], F32)  # (b16, d)
        L_t = cpool.tile([128, B16 * CIN], F32)  # logits (b16, c)
        Ltmp = cpool.tile([128, B16 * CIN], F32)
        E_t = cpool.tile([128, B16 * CIN], BF16)
        C_t = cpool.tile([128, B16 * CIN], BF16)
        Dh = cpool.tile([128, CIN], BF16)
        Dtmp = cpool.tile([128, CIN], BF16)
        Dt8 = cpool.tile([128, 8 * CIN], BF16)
        Rh = cpool.tile([128, CIN], BF16)
        sq = cpool.tile([128, 256], F32)
        n2 = cpool.tile([128, B16], F32)
        t1 = cpool.tile([128, B16], F32)
        r1 = cpool.tile([128, B16], F32)
        rs = cpool.tile([128, B16], F32)
        fac = cpool.tile([128, B16], F32)
        outB = cpool.tile([128, 256], BF16)
        outB2 = cpool.tile([128, 512], BF16)  # outB duplicated per c-pair
        outF = cpool.tile([128, 256], F32)
        epsb = cpool.tile([128, 1], F32)
        nc.vector.memset(epsb[:], EPS)

        for r in range(4):
            _e = nc.sync if os.environ.get("SYNC_DMA_ONLY") else nc.scalar
            _e.dma_start(ilhs_t[32 * r : 32 * r + 8, :], ilhs_d[8 * r : 8 * r + 8, :])

        H1av = H1a[:].rearrange("p (b d c) -> p b d c", b=B16, d=D)
        H1bv = H1b[:].rearrange("p (b d c) -> p b d c", b=B16, d=D)

        def hslice(c0, c1):
            # view of H columns [c0, c1) — must not cross c=72
            if c1 <= 72:
                return H1av[:, :, :, c0:c1]
            assert c0 >= 72
            return H1bv[:, :, :, c0 - 72 : c1 - 72]

        # ---- H generation ----
        with (
            tc.tile_pool(name="w", bufs=2) as wpool,
            tc.tile_pool(name="psum", bufs=4, space="PSUM") as pp,
        ):
            # spread the W row-group loads across two issuing engines so the
            # transfers run in parallel instead of serializing on one queue,
            # and stream in 4 chunks so matmuls start early
            if os.environ.get("SYNC_DMA_ONLY"):
                dma_eng = [nc.sync, nc.sync, nc.sync, nc.sync]
            else:
                dma_eng = [nc.sync, nc.sync, nc.gpsimd, nc.gpsimd]
            CHW = 9 * 512
            for h in range(4):
                wc = wpool.tile([128, CHW], BF16)
                for r in range(4):
                    dma_eng[r].dma_start(
                        wc[32 * r : 32 * r + 8, :],
                        wrhs_d[8 * r : 8 * r + 8, h * CHW : (h + 1) * CHW],
                    )
                for cql in range(9):
                    cq = h * 9 + cql
                    lhs = ilhs_t[:, cq * BS : (cq + 1) * BS]
                    # one [128,256] PSUM tile per c, partition-split by the
                    # two col-group matmuls. (Column-splitting a PSUM bank
                    # between matmuls crashes the device - start=True clears
                    # the whole bank, so keep one c per bank.)
                    for r in range(4):
                        c = 4 * cq + r
                        pts = pp.tile([128, 256], F32, tag="ptsm")
                        for b0 in range(2):
                            rhs = wc[
                                32 * r : 32 * r + 8,
                                cql * 512 + b0 * 256 : cql * 512 + b0 * 256 + 256,
                            ]
                            nc.tensor.matmul(
                                pts[b0 * 64 : (b0 + 1) * 64, :],
                                lhs[32 * r : 32 * r + 8, :],
                                rhs,
                                start=True,
                                stop=True,
                                tile_position=(32 * r, b0 * 64),
                            )
                        dsts = hslice(c, c + 1).squeeze(3)
                        srcs = pts[:].rearrange("p (b d) -> p b d", b=B16)
                        if r % 2 == 0:
                            nc.vector.tensor_copy(dsts, srcs)
                        else:
                            nc.scalar.copy(dsts, srcs)
        # ---- routing ----
        if True:
            prodv = prod[:].rearrange("p (b d c) -> p b d c", b=B16, d=D)

            s_v = s_t[:].rearrange("p (b d) -> p b d", b=B16)
            L_v = L_t[:].rearrange("p (b c) -> p b c", b=B16)
            Lt_v = Ltmp[:].rearrange("p (b c) -> p b c", b=B16)
            E_v = E_t[:].rearrange("p (b c) -> p b c", b=B16)
            C_v = C_t[:].rearrange("p (b c) -> p b c", b=B16)
            outB_v = outB[:].rearrange("p (b d) -> p b d", b=B16)

            V = nc.vector
            P = nc.vector if os.environ.get("NO_POOL_TT") else nc.gpsimd

            def tree_add(eng, lo, width):
                # halve [lo, lo+width) in place until width 3 (pool) / 6 (dve)
                w = width
                while w % 2 == 0 and w > (6 if eng is V else 3):
                    w //= 2
                    eng.tensor_tensor(
                        prodv[:, :, :, lo : lo + w],
                        prodv[:, :, :, lo : lo + w],
                        prodv[:, :, :, lo + w : lo + 2 * w],
                        op=ALU.add,
                    )
                return w

            def pool_fold(lo, w, eng=None):
                # fold w columns starting at lo into column lo
                eng = eng or P
                while w > 1:
                    half = w // 2
                    eng.tensor_tensor(
                        prodv[:, :, :, lo : lo + half],
                        prodv[:, :, :, lo : lo + half],
                        prodv[:, :, :, lo + half : lo + 2 * half],
                        op=ALU.add,
                    )
                    if w % 2 == 1:
                        eng.tensor_tensor(
                            prodv[:, :, :, lo : lo + 1], prodv[:, :, :, lo : lo + 1],
                            prodv[:, :, :, lo + w - 1 : lo + w], op=ALU.add,
                        )
                    w = half

            def s0_phase():
                # uniform coefficients: s0 = (1/32) sum_c H. Runs as 4 pool
                # chunk-trees matching H-generation order, so most of it hides
                # under the tail of the H-drain copies.
                for k in range(4):
                    lo = 36 * k
                    eng = P if k < 2 else V
                    eng.tensor_tensor(
                        prodv[:, :, :, lo : lo + 18],
                        hslice(lo, lo + 18),
                        hslice(lo + 18, lo + 36),
                        op=ALU.add,
                    )
                    eng.tensor_tensor(
                        prodv[:, :, :, lo : lo + 9],
                        prodv[:, :, :, lo : lo + 9],
                        prodv[:, :, :, lo + 9 : lo + 18],
                        op=ALU.add,
                    )
                    pool_fold(lo, 9, eng=eng)
                V.tensor_tensor(
                    prodv[:, :, :, 0:1], prodv[:, :, :, 0:1],
                    prodv[:, :, :, 36:37], op=ALU.add,
                )
                V.tensor_tensor(
                    prodv[:, :, :, 72:73], prodv[:, :, :, 72:73],
                    prodv[:, :, :, 108:109], op=ALU.add,
                )
                V.tensor_tensor(
                    prodv[:, :, :, 0:1], prodv[:, :, :, 0:1],
                    prodv[:, :, :, 72:73], op=ALU.add,
                )
                V.tensor_scalar(
                    s_v, prodv[:, :, :, 0:1].squeeze(3), 1.0 / NCAP, None,
                    op0=ALU.mult,
                )

            def s_phase():
                # DVE owns c [0, 88), Pool owns [88, 144)
                cb = C_v.unsqueeze(2).broadcast_to((128, B16, D, CIN))
                V.tensor_tensor(
                    prodv[:, :, :, 0:72], hslice(0, 72),
                    cb[:, :, :, 0:72], op=ALU.mult,
                )
                V.tensor_tensor(
                    prodv[:, :, :, 72:88], hslice(72, 88),
                    cb[:, :, :, 72:88], op=ALU.mult,
                )
                P.tensor_tensor(
                    prodv[:, :, :, 88:144], hslice(88, 144),
                    cb[:, :, :, 88:144], op=ALU.mult,
                )
                # DVE tree 88 -> 44 -> 22 -> (10 pairs + 2 spill) -> reduce-10
                V.tensor_tensor(
                    prodv[:, :, :, 0:44], prodv[:, :, :, 0:44],
                    prodv[:, :, :, 44:88], op=ALU.add,
                )
                V.tensor_tensor(
                    prodv[:, :, :, 0:22], prodv[:, :, :, 0:22],
                    prodv[:, :, :, 22:44], op=ALU.add,
                )
                V.tensor_tensor(
                    prodv[:, :, :, 0:10], prodv[:, :, :, 0:10],
                    prodv[:, :, :, 10:20], op=ALU.add,
                )
                V.tensor_tensor(
                    prodv[:, :, :, 0:2], prodv[:, :, :, 0:2],
                    prodv[:, :, :, 20:22], op=ALU.add,
                )
                # pool tree 56 -> 28 -> 14 -> 7 -> fold
                P.tensor_tensor(
                    prodv[:, :, :, 88:116], prodv[:, :, :, 88:116],
                    prodv[:, :, :, 116:144], op=ALU.add,
                )
                P.tensor_tensor(
                    prodv[:, :, :, 88:102], prodv[:, :, :, 88:102],
                    prodv[:, :, :, 102:116], op=ALU.add,
                )
                P.tensor_tensor(
                    prodv[:, :, :, 88:95], prodv[:, :, :, 88:95],
                    prodv[:, :, :, 95:102], op=ALU.add,
                )
                pool_fold(88, 7)
                V.reduce_sum(s_v, prodv[:, :, :, 0:10], axis=AX.X)
                V.tensor_tensor(
                    s_v, s_v, prodv[:, :, :, 88:89].squeeze(3), op=ALU.add
                )

            def squash(final: bool):
                nc.vector.tensor_tensor(sq[:], s_t[:], s_t[:], op=ALU.mult)
                nc.vector.reduce_sum(
                    n2[:], sq[:].rearrange("p (b d) -> p b d", b=B16), axis=AX.X
                )
                nc.scalar.activation(rs[:], n2[:], AF.Sqrt, bias=epsb[:])
                # t1 = (n2 + 1) * sqrt(n2 + eps) in one fused op
                nc.vector.scalar_tensor_tensor(
                    t1[:], n2[:], 1.0, rs[:], op0=ALU.add, op1=ALU.mult
                )
                nc.vector.reciprocal(r1[:], t1[:])
                nc.vector.tensor_tensor(fac[:], n2[:], r1[:], op=ALU.mult)
                facb = fac[:].unsqueeze(2).broadcast_to((128, B16, D))
                if final:
                    nc.vector.tensor_tensor(outF[:].rearrange("p (b d) -> p b d", b=B16), s_v, facb, op=ALU.mult)
                else:
                    nc.vector.tensor_tensor(outB_v, s_v, facb, op=ALU.mult)

            H1abd = H1a[:].rearrange("p (bd c) -> p bd c", c=72)
            H1bbd = H1b[:].rearrange("p (bd c) -> p bd c", c=72)
            prodbd = prod[:].rearrange("p (bd c) -> p bd c", c=CIN)
            outB2v = outB2[:].rearrange("p (bd c2) -> p bd c2", c2=2)

            def b_update(first: bool):
                # expand outB so each value appears for a c-PAIR: the multiply
                # operand then has an innermost [step=1, count=2] dim, which
                # keeps the DVE in 2x packed mode (a plain broadcast would
                # have innermost step 0 -> 1x). Pool is idle here, let it do
                # the tiny expansion.
                P.tensor_copy(
                    outB2v,
                    outB[:].unsqueeze(2).broadcast_to((128, 256, 2)),
                )
                # DVE owns c [0, 88), Pool owns [88, 144)
                ranges = [(V, 0, 72), (V, 72, 88), (P, 88, 144)]
                for eng, lo, hi in ranges:
                    npair = (hi - lo) // 2
                    hb = H1abd if hi <= 72 else H1bbd
                    ho = lo if hi <= 72 else lo - 72
                    h_in = hb[:, :, ho : ho + (hi - lo)].rearrange(
                        "p bd (cp c2) -> p bd cp c2", c2=2
                    )
                    o_in = outB2v.unsqueeze(2).broadcast_to((128, 256, npair, 2))
                    p_out = prodbd[:, :, lo:hi].rearrange(
                        "p bd (cp c2) -> p bd cp c2", c2=2
                    )
                    eng.tensor_tensor(p_out, h_in, o_in, op=ALU.mult)
                    for w in (8, 4, 2):
                        eng.tensor_tensor(
                            prodv[:, :, 0:w, lo:hi],
                            prodv[:, :, 0:w, lo:hi],
                            prodv[:, :, w : 2 * w, lo:hi],
                            op=ALU.add,
                        )
                    d0 = prodv[:, :, 0:1, lo:hi].squeeze(2)
                    d1 = prodv[:, :, 1:2, lo:hi].squeeze(2)
                    if first:
                        eng.tensor_tensor(L_v[:, :, lo:hi], d0, d1, op=ALU.add)
                    else:
                        eng.tensor_tensor(Lt_v[:, :, lo:hi], d0, d1, op=ALU.add)
                        eng.tensor_tensor(
                            L_v[:, :, lo:hi], L_v[:, :, lo:hi],
                            Lt_v[:, :, lo:hi], op=ALU.add,
                        )

            def softmax():
                # c-range split so exp on the DVE-owned logits range starts
                # as soon as DVE's L-add is done (pool's range still in
                # flight), and the two denominator trees run on different
                # engines in parallel.
                Dt8v = Dt8[:].rearrange("p (b c) -> p b c", b=8)
                for lo, hi, teng in ((0, 88, V), (88, CIN, P)):
                    nc.scalar.activation(
                        E_v[:, :, lo:hi], L_v[:, :, lo:hi], AF.Exp
                    )
                    teng.tensor_tensor(
                        Dt8v[:, 0:8, lo:hi], E_v[:, 0:8, lo:hi],
                        E_v[:, 8:16, lo:hi], op=ALU.add,
                    )
                    teng.tensor_tensor(
                        Dt8v[:, 0:4, lo:hi], Dt8v[:, 0:4, lo:hi],
                        Dt8v[:, 4:8, lo:hi], op=ALU.add,
                    )
                    teng.tensor_tensor(
                        Dt8v[:, 0:2, lo:hi], Dt8v[:, 0:2, lo:hi],
                        Dt8v[:, 2:4, lo:hi], op=ALU.add,
                    )
                    teng.tensor_tensor(
                        Dh[:, lo:hi].unsqueeze(1), Dt8v[:, 0:1, lo:hi],
                        Dt8v[:, 1:2, lo:hi], op=ALU.add,
                    )
                # swap halves with two parallel DMAs, then full-width add+recip
                # (both partition halves end up with the full denominator)
                nc.sync.dma_start(Dtmp[0:64, :], Dh[64:128, :])
                _e2 = nc.sync if os.environ.get("SYNC_DMA_ONLY") else nc.gpsimd
                _e2.dma_start(Dtmp[64:128, :], Dh[0:64, :])
                nc.vector.tensor_tensor(Dh[:], Dh[:], Dtmp[:], op=ALU.add)
                with nc.allow_low_precision(
                    reason="softmax coefficients are bf16 throughout"
                ):
                    nc.vector.reciprocal(Rh[:], Dh[:])
                rb = Rh[:].unsqueeze(1).broadcast_to((128, B16, CIN))
                V.tensor_tensor(
                    C_v[:, :, 0:88], E_v[:, :, 0:88], rb[:, :, 0:88], op=ALU.mult
                )
                P.tensor_tensor(
                    C_v[:, :, 88:CIN], E_v[:, :, 88:CIN], rb[:, :, 88:CIN],
                    op=ALU.mult,
                )

            for it in range(ROUTINGS):
                if it == 0:
                    s0_phase()
                else:
                    s_phase()
                squash(final=(it == ROUTINGS - 1))
                if it < ROUTINGS - 1:
                    b_update(first=(it == 0))
                    softmax()

            for b0 in range(2):
                oap = out_d[:, b0 * B16 : (b0 + 1) * B16, :].rearrange(
                    "a b d -> a (b d)"
                )
                nc.sync.dma_start(oap, outF[b0 * 64 : (b0 + 1) * 64, :])

    # The TRN2 matmul ISA encoding only fits one sync wait; Tile can emit
    # several. Run the bacc fix-up passes: excess matmul waits move to the
    # paired ldweights, and any instruction still holding >1 wait gets them
    # split into preceding EventSemaphore instructions.
    import bass_rust as _bass_rust

    _bass_rust.move_matmul_waits_to_ldweights(nc.m)
    _bass_rust.generate_event_semaphores(nc)
    return nc


def _bf16(x: np.ndarray) -> np.ndarray:
    import ml_dtypes

    return x.astype(ml_dtypes.bfloat16)


def _pack_w(W: np.ndarray) -> np.ndarray:
    # wrhs[8r+i, cq*512 + b*16 + j] = W[b, 4cq+r, j, i]
    wrhs = np.empty((32, CQ * 512), np.float32)
    for r in range(4):
        blk = W[:, r::4, :, :]  # [b, cq, j, i]
        wrhs[8 * r : 8 * r + 8, :] = np.ascontiguousarray(
            blk.transpose(3, 1, 0, 2)
        ).reshape(8, CQ * 512)
    return _bf16(wrhs)


def _pack_x(xs: np.ndarray) -> np.ndarray:
    # ilhs[8r+i, cq*64 + a] = xs[a, 4cq+r, i]
    ilhs = np.empty((32, CQ * BS), np.float32)
    for r in range(4):
        blk = xs[:, r::4, :]  # [a, cq, i]
        ilhs[8 * r : 8 * r + 8, :] = np.ascontiguousarray(blk.transpose(2, 1, 0)).reshape(
            8, CQ * BS
        )
    return _bf16(ilhs)


_CACHED = {}


def _get_program():
    if "nc" not in _CACHED:
        _CACHED["nc"] = _build_program()
    return _CACHED["nc"]


def kernel(inputs: np.ndarray, W: np.ndarray) -> np.ndarray:
    inputs = np.asarray(inputs, np.float32)
    W = np.asarray(W, np.float32)
    nc = _get_program()
    wrhs = _pack_w(W)
    in_maps = []
    for k in range(NCORES):
        xs = inputs[k * BS : (k + 1) * BS]
        in_maps.append({"ilhs": _pack_x(xs), "wrhs": wrhs})
    res = run_bass_kernel_spmd(nc, in_maps, core_ids=list(range(NCORES)))
    out = np.concatenate([res.results[k]["out"] for k in range(NCORES)], axis=0)
    return out

